# revision 16
# baseline (speedup 1.0000x reference)
"""Multi-head attention (B=2, N=2048, D=1024, H=16) on 8 Trainium2 cores.

Sharding: data-parallel over batch (2) x tensor-parallel over head groups (4).
Core c handles batch c//4, heads 4*(c%4) .. 4*(c%4)+3.

Per-core kernel (matmuls at full PE rate via float32r, P*V in bf16):
  front:   kT = ([Wk;bk]^T @ [x^T;1])   (channels on partitions)
           v  = ([x;1] @ [Wv;bv])       (tokens on partitions, [v|1] blocks)
  per query-tile of 512 (ACT-exp is the pacing engine, ~32us/tile):
           qT slice = ([Wq;bq]^T @ [x^T;1])
           for each key-ptile of 128:
             S^T[:,h,:] = kT_h^T qT_h    (keys on partitions, 4 single-shot
                                          matmuls into the 4 banks of one tile)
             P^T = exp(SCALE * S^T)      (one ACT op over all 4 heads)
             [O^T_h; sums_h] += [v_h|1]^T P^T_h   (per-head chain, own bank)
           O^T_h *= broadcast(1/sums_h)  (DVE recip + gpsimd bcast + DVE mul)
           out[tokens of this tile] = sum_h O^T_h^T @ Wo_h  (K=64 chains)
Host: out[b] = sum of the 4 group partials + b_o.
"""

import sys

sys.path.insert(0, "/opt/trn_rl_repo")

import numpy as np

B, N, D, H = 2, 2048, 1024, 16
SUB = D // H  # 64
GROUPS = 4  # tensor-parallel head groups
NH = H // GROUPS  # 4 local heads per core
CH = NH * SUB  # 256 local channels
NCORES = 8


def build_nc(NT=N, DK=D, DO=D, nh=NH, name="mha"):
    import concourse.mybir as mybir
    from concourse import bacc
    from concourse.tile import TileContext

    f32 = mybir.dt.float32
    f32r = mybir.dt.float32r
    bf16 = mybir.dt.bfloat16
    Exp = mybir.ActivationFunctionType.Exp
    mult = mybir.AluOpType.mult

    sub = 64
    ch = nh * sub
    KT = DK // 128  # contraction ptiles
    CHT = ch // 128  # channel ptiles
    TOKT = NT // 128  # token/key ptiles
    QT = NT // 512  # query tiles
    TPQ = TOKT // QT  # token ptiles emitted per query tile (4)
    scale = sub ** -0.5

    nc = bacc.Bacc(None, name=name)
    xT = nc.dram_tensor("xT", [DK, NT], f32r, kind="ExternalInput")
    wq = nc.dram_tensor("wq", [DK + 1, ch], f32r, kind="ExternalInput")
    wk = nc.dram_tensor("wk", [DK + 1, ch], f32r, kind="ExternalInput")
    wv = nc.dram_tensor("wv", [DK + 1, nh * 65], f32r, kind="ExternalInput")
    wo = nc.dram_tensor("wo", [ch, DO], f32r, kind="ExternalInput")
    ones_in = nc.dram_tensor("ones", [1, 512], f32r, kind="ExternalInput")
    out = nc.dram_tensor("out", [NT, DO], f32, kind="ExternalOutput")

    with TileContext(nc) as tc:
        with tc.tile_pool(name="persist", bufs=1) as pp:
            ones = pp.tile([1, 512], f32r)
            qT_sb = pp.tile([128, CHT, NT], f32r)
            kT_sb = pp.tile([128, CHT, NT], f32r)
            v_sb = pp.tile([128, TOKT, nh * 65], bf16)  # [v_h | 1] blocks
            oT_sb = pp.tile([128, CHT, NT], f32r)
            wo_sb = pp.tile([128, CHT, DO], f32r)
            nc.sync.dma_start(ones[:], ones_in[:])
            for ct in range(CHT):
                nc.sync.dma_start(wo_sb[:, ct, :], wo[ct * 128 : (ct + 1) * 128, :])

            with tc.tile_pool(name="xp", bufs=1) as xp, \
                 tc.tile_pool(name="wp", bufs=1) as wp, \
                 tc.tile_pool(name="stp", bufs=2, space="PSUM") as stp, \
                 tc.tile_pool(name="acc", bufs=4, space="PSUM") as acc, \
                 tc.tile_pool(name="ptp", bufs=6) as ptp, \
                 tc.tile_pool(name="nrm", bufs=3) as nrm, \
                 tc.tile_pool(name="osg", bufs=2) as osg:
                xt = xp.tile([128, KT, NT], f32r)
                w_sb = {}

                def load_w(nm, dram):
                    wch = nh * 65 if nm == "wv" else ch
                    wt = wp.tile([128, KT, wch], f32r, name=f"{nm}t", tag=f"{nm}t")
                    for kt in range(KT):
                        nc.sync.dma_start(
                            wt[:, kt, :], dram[kt * 128 : (kt + 1) * 128, :]
                        )
                    wb = wp.tile([1, wch], f32r, name=f"{nm}b", tag=f"{nm}b")
                    nc.sync.dma_start(wb[:], dram[DK : DK + 1, :])
                    w_sb[nm] = (wt, wb)

                load_w("wk", wk)
                for kt in range(KT):
                    nc.sync.dma_start(xt[:, kt, :], xT[kt * 128 : (kt + 1) * 128, :])
                load_w("wv", wv)
                load_w("wq", wq)

                def qk_proj(dst, nm, mt, qt, pool=None, tag="acc"):
                    """dst[:, mt, qt*512:+512] = ([W;b]^T @ [x^T;1]) slice."""
                    wt, wb = w_sb[nm]
                    ps = (pool or acc).tile([128, 512], f32, name="ps", tag=tag)
                    for kt in range(KT):
                        nc.tensor.matmul(
                            ps[:],
                            lhsT=wt[:, kt, mt * 128 : (mt + 1) * 128],
                            rhs=xt[:, kt, qt * 512 : (qt + 1) * 512],
                            start=(kt == 0),
                            stop=False,
                        )
                    nc.tensor.matmul(
                        ps[:],
                        lhsT=wb[0:1, mt * 128 : (mt + 1) * 128],
                        rhs=ones[0:1, 0:512],
                        start=False,
                        stop=True,
                    )
                    nc.vector.tensor_copy(dst[:, mt, qt * 512 : (qt + 1) * 512], ps[:])

                def v_proj(tt):
                    """v_sb[:, tt, 65h:65h+64] = ([x;1] @ [Wv;bv]) block; col 65h+64 = 1."""
                    wt, wb = w_sb["wv"]
                    ps = acc.tile([128, nh * 65], f32, name="psv", tag="acc")
                    for kt in range(KT):
                        nc.tensor.matmul(
                            ps[:],
                            lhsT=xt[:, kt, tt * 128 : (tt + 1) * 128],
                            rhs=wt[:, kt, :],
                            start=(kt == 0),
                            stop=False,
                        )
                    nc.tensor.matmul(
                        ps[:],
                        lhsT=ones[0:1, 0:128],
                        rhs=wb[:],
                        start=False,
                        stop=True,
                    )
                    nc.vector.tensor_copy(v_sb[:, tt, :], ps[:])

                def outproj_piece(tt, nt):
                    ps = acc.tile([128, 512], f32, name="ops", tag="acc")
                    for ct in range(CHT):
                        nc.tensor.matmul(
                            ps[:],
                            lhsT=oT_sb[:, ct, tt * 128 : (tt + 1) * 128],
                            rhs=wo_sb[:, ct, nt * 512 : (nt + 1) * 512],
                            start=(ct == 0),
                            stop=(ct == CHT - 1),
                        )
                    stg = osg.tile([128, 512], f32, name="stg", tag="stg")
                    nc.vector.tensor_copy(stg[:], ps[:])
                    nc.sync.dma_start(
                        out[tt * 128 : (tt + 1) * 128, nt * 512 : (nt + 1) * 512],
                        stg[:],
                    )

                # front: full kT, v, qT.  k-chain and v-chain matmuls are
                # interleaved so vproj's per-matmul LDWEIGHTS (stationary =
                # x tiles) hides under kproj's 512-wide streams.
                wkt, wkb = w_sb["wk"]
                wvt, wvb = w_sb["wv"]
                for i in range(CHT * QT):
                    mt, qt = divmod(i, QT)
                    psk = acc.tile([128, 512], f32, name="psk", tag="acc")
                    psa = acc.tile([128, nh * 65], f32, name="psa", tag="acc")
                    psb = acc.tile([128, nh * 65], f32, name="psb", tag="acc")
                    for kt in range(KT):
                        nc.tensor.matmul(
                            psk[:],
                            lhsT=wkt[:, kt, mt * 128 : (mt + 1) * 128],
                            rhs=xt[:, kt, qt * 512 : (qt + 1) * 512],
                            start=(kt == 0), stop=False,
                        )
                        for ps_, tt in ((psa, 2 * i), (psb, 2 * i + 1)):
                            nc.tensor.matmul(
                                ps_[:],
                                lhsT=xt[:, kt, tt * 128 : (tt + 1) * 128],
                                rhs=wvt[:, kt, :],
                                start=(kt == 0), stop=False,
                            )
                    nc.tensor.matmul(
                        psk[:], lhsT=wkb[0:1, mt * 128 : (mt + 1) * 128],
                        rhs=ones[0:1, 0:512], start=False, stop=True,
                    )
                    for ps_, tt in ((psa, 2 * i), (psb, 2 * i + 1)):
                        nc.tensor.matmul(
                            ps_[:], lhsT=ones[0:1, tt * 128 : (tt + 1) * 128],
                            rhs=wvb[:], start=False, stop=True,
                        )
                    nc.vector.tensor_copy(kT_sb[:, mt, qt * 512 : (qt + 1) * 512], psk[:])
                    nc.vector.tensor_copy(v_sb[:, 2 * i, :], psa[:])
                    nc.vector.tensor_copy(v_sb[:, 2 * i + 1, :], psb[:])
                for mt in range(CHT):
                    qk_proj(qT_sb, "wq", mt, 0)

                for qt in range(QT):
                    ot = [
                        acc.tile([65, 512], f32, name=f"ot{h}", tag="acc")
                        for h in range(nh)
                    ]
                    for kt2 in range(TOKT):
                        first, last = kt2 == 0, kt2 == TOKT - 1
                        # two half-tiles (2 heads / 2 banks each), double-buffered:
                        # exp of one half pipelines against S-matmuls of the other
                        for half in range(nh // 2):
                            st = stp.tile([128, 2, 512], f32, name="st", tag="st")
                            for hh in range(2):
                                h = 2 * half + hh
                                bp = 64 * hh
                                nc.tensor.matmul(
                                    st[:, hh, :],
                                    lhsT=kT_sb[bp : bp + 64, half, kt2 * 128 : (kt2 + 1) * 128],
                                    rhs=qT_sb[bp : bp + 64, half, qt * 512 : (qt + 1) * 512],
                                    start=True,
                                    stop=True,
                                )
                            pt = ptp.tile([128, 2, 512], bf16, name="pt", tag="pt")
                            nc.scalar.activation(pt[:], st[:], Exp, scale=scale)
                            for hh in range(2):
                                h = 2 * half + hh
                                nc.tensor.matmul(
                                    ot[h][:],
                                    lhsT=v_sb[:, kt2, 65 * h : 65 * h + 65],
                                    rhs=pt[:, hh, :],
                                    start=first,
                                    stop=last,
                                )
                    if qt + 1 < QT:
                        for mt in range(CHT):
                            qk_proj(qT_sb, "wq", mt, qt + 1, pool=stp, tag="st")
                    for h in range(nh):
                        bp = 64 * (h % 2)
                        rcp = nrm.tile([65, 512], f32, name="rcp", tag="rcp")
                        row0 = nrm.tile([1, 512], f32, name="row0", tag="row0")
                        bc = nrm.tile([64, 512], f32, name="bc", tag="bc")
                        nc.vector.reciprocal(rcp[64:65, :], ot[h][64:65, :])
                        # gpsimd broadcast reads physical partition 0: stage there
                        nc.sync.dma_start(row0[:], rcp[64:65, :])
                        nc.gpsimd.partition_broadcast(bc[:], row0[:], channels=64)
                        nc.vector.tensor_tensor(
                            out=oT_sb[bp : bp + 64, h // 2, qt * 512 : (qt + 1) * 512],
                            in0=ot[h][0:64, :],
                            in1=bc[:],
                            op=mult,
                        )
                    for tt in range(qt * TPQ, min((qt + 1) * TPQ, TOKT)):
                        for nt in range(DO // 512):
                            outproj_piece(tt, nt)
    nc.finalize()
    return nc


def _augment_wv(Wv, bv):
    """(D, 256) + (256,) -> (D+1, 260): per head [Wv_h | e_ones], bias row [bv_h | 1]."""
    DK = Wv.shape[0]
    out = np.zeros((DK + 1, NH * 65), dtype=np.float32)
    for h in range(NH):
        out[:DK, 65 * h : 65 * h + 64] = Wv[:, 64 * h : 64 * h + 64]
        out[DK, 65 * h : 65 * h + 64] = bv[64 * h : 64 * h + 64]
        out[DK, 65 * h + 64] = 1.0
    return out


def make_in_maps(x, W_qkv, b_qkv, W_o):
    """Shard full inputs into per-core input maps (core c: batch c//4, group c%4)."""
    x = np.asarray(x, dtype=np.float32)
    W_qkv = np.asarray(W_qkv, dtype=np.float32)
    b_qkv = np.asarray(b_qkv, dtype=np.float32)
    W_o = np.asarray(W_o, dtype=np.float32)
    in_maps = []
    for c in range(NCORES):
        b, g = divmod(c, GROUPS)
        cols = slice(CH * g, CH * (g + 1))
        m = {
            "xT": np.ascontiguousarray(x[b].T),
            "wq": np.ascontiguousarray(
                np.concatenate([W_qkv[:, 0 * D : 1 * D][:, cols], b_qkv[0 * D : 1 * D][cols][None, :]], 0)
            ),
            "wk": np.ascontiguousarray(
                np.concatenate([W_qkv[:, 1 * D : 2 * D][:, cols], b_qkv[1 * D : 2 * D][cols][None, :]], 0)
            ),
            "wv": _augment_wv(
                W_qkv[:, 2 * D : 3 * D][:, cols], b_qkv[2 * D : 3 * D][cols]
            ),
            "wo": np.ascontiguousarray(W_o[cols, :]),
            "ones": np.ones((1, 512), dtype=np.float32),
        }
        in_maps.append(m)
    return in_maps


_NC = None


def get_nc():
    global _NC
    if _NC is None:
        _NC = build_nc()
    return _NC


def kernel(x, W_qkv, b_qkv, W_o, b_o):
    from concourse import bass_utils

    b_o = np.asarray(b_o, dtype=np.float32)
    in_maps = make_in_maps(x, W_qkv, b_qkv, W_o)
    res = bass_utils.run_bass_kernel_spmd(get_nc(), in_maps, core_ids=list(range(NCORES)))
    out = np.empty((B, N, D), dtype=np.float32)
    for b in range(B):
        acc = res.results[4 * b]["out"].copy()
        for g in range(1, GROUPS):
            acc += res.results[4 * b + g]["out"]
        out[b] = acc + b_o
    return out
